# revision 7
# baseline (speedup 1.0000x reference)
"""MoE-routed DeepQNetwork kernel for 8x Trainium2 NeuronCores.

Problem: B=65536 rows, each routed to one of E=8 expert MLPs
(256 -> 64 -> 64 -> 64 -> 64 -> 64 -> 18, ReLU between layers).

Strategy (expert-grouped sharding, v4):
  Host: stable-sort rows by expert, pad each expert group to a multiple of
  1024 rows so every PAIR of 512-row blocks is single-expert, split the
  sorted+padded batch into 8 equal per-core chunks. Each core runs a static
  expert-agnostic program; expert identity is carried in per-core weight /
  bias tensors (per-pair compact layout, 676 fp16 cols each).

  Device (per core, SPMD): pairs advance two-at-a-time ("duos") through a
  diagonal duo/layer wavefront.  Each layer of a duo lands in a shared
  [128,1024] fp32 PSUM tile (two [*,512] matmuls, one per 512-fp32 bank) so
  ONE ReLU+bias op drains both pairs -> the Vector/Scalar engines (the
  steady-state pacers otherwise) run below the PE's ~2.15us/pair pace and
  the PE stays continuously busy at the 2.4GHz p-state (idle PE throttles
  to 1.2/0.65GHz; dummy matmuls over a scratch tile warm it up while the
  first inputs are in flight).  L1 runs per-block on PE column groups
  (M=64); L2-5 as [128,128] block-diag; L6 as [128,36] block-diag into a
  [36,1024] PSUM tile copied out once per duo (b6 is added on the host).

  DMA: every dma_start costs ~600-900ns of issue time on its sequencer and
  the 16 DMA queues drain strictly FIFO, so transfers are issued in NEED
  order round-robin across the three DMA-capable engines (sync/SP,
  scalar/Activation, gpsimd/Pool): w(pair0), x0a, x0b, bias, w(1-2), x1,
  x2, ... and outputs go out on sync as each duo finishes.

  Host: unsort the [36, rows] fp16 outputs back to original row order, +b6.
"""

import math
import os

import numpy as np

E = 8
D = 256
H = 64
A = 18
NCORES = 8
BLK = 512  # rows per block (matmul moving-operand free dim)
PAIR = 2 * BLK  # rows per pair; expert groups padded to this granularity

# per-pair fp16 weight tensor column layout (WCOLS = 676):
#   [0:64)    W1 K-half0  [128, 64]
#   [64:128)  W1 K-half1  [128, 64]
#   [128:640) W2..W5, each [128, 128] block-diag: [0:64, 0:64] = W_l[e],
#             [64:128, 64:128] = W_l[e]
#   [640:676) W6 [128, 36] block-diag: [0:64, 0:18] = W6[e],
#             [64:128, 18:36] = W6[e]
WCOLS = 676

_PROGRAM_CACHE: dict = {}
LAST_RESULTS = None  # test harness can read timing/profile info from here


def _build_program(npair: int, duo_same: tuple):
    """SPMD bass program: npair single-expert 1024-row pairs, processed as
    duos; duo_same[d] means both pairs of duo d share one expert on EVERY
    core (single bias per ReLU op is then valid)."""
    import concourse.mybir as mybir
    import concourse.tile as tile
    from concourse import bacc

    f32 = mybir.dt.float32
    f16 = mybir.dt.float16
    Relu = mybir.ActivationFunctionType.Relu
    add = mybir.AluOpType.add
    amax = mybir.AluOpType.max

    nduo = (npair + 1) // 2

    nc = bacc.Bacc("TRN2")
    xall = nc.declare_dram_parameter("xall", [128, npair * 2048], f16, isOutput=False)
    wall = nc.declare_dram_parameter("wall", [128, npair * WCOLS], f16, isOutput=False)
    # per pair: cols 0:5 = b1..b5 (rows 0:64 = rows 64:128 = bias of the
    # pair's expert); b6 is added on the host.
    bias = nc.declare_dram_parameter("bias", [128, npair * 5], f32, isOutput=False)
    yt = nc.declare_dram_parameter("yt", [36, npair * BLK], f16, isOutput=True)

    # weight DMA chunks (pair ranges): pair 0 alone so L1(0) starts early
    wchunks = [(0, 1)] + [(p, min(p + 2, npair)) for p in range(1, npair, 2)]

    with tile.TileContext(nc) as tc:
        with (
            tc.tile_pool(name="wpool", bufs=1) as wpool,
            tc.tile_pool(name="xpool", bufs=npair + 1) as xpool,
            tc.tile_pool(name="hpool", bufs=3) as hpool,
            tc.tile_pool(name="opool", bufs=3) as opool,
            tc.tile_pool(name="ppool", bufs=2, space="PSUM") as ppool,
            tc.tile_pool(name="popool", bufs=2, space="PSUM") as popool,
        ):
            # ---- PE warm-up: the Tensor engine p-state reaches 2.4GHz only
            # after ~3us of continuous busy.  Burn the dead time before the
            # first x/w tiles land on dummy matmuls over a scratch tile
            # (memset on the otherwise-idle Vector engine) so the real
            # stream starts at full clock.
            scratch = wpool.tile([128, BLK], f16, tag="scr", name="scratch", bufs=1)
            nc.vector.memset(scratch[:, :], 0.0)
            for d in range(10):
                pd = ppool.tile([128, 1024], f32, tag="ph", name=f"pdummy_{d}")
                nc.tensor.matmul(
                    out=pd[0:64, 0:BLK],
                    lhsT=scratch[:, 0:64],
                    rhs=scratch[:, :],
                    start=True,
                    stop=True,
                )

            # ---- input DMAs in need order, round-robin across the three
            # DMA-capable sequencers so issue cost doesn't serialize and the
            # FIFO queues hand tiles over in consumption order.
            wtiles = {}
            xts = [None] * npair
            x0ab = [None, None]
            bias_sb = None
            dma_ops = []  # (kind, arg)
            for ci, (p0, p1) in enumerate(wchunks):
                dma_ops.append(("w", ci))
                if ci == 0:
                    dma_ops += [("x0a", 0), ("x0b", 0), ("bias", 0)]
                else:
                    dma_ops += [("x", p) for p in range(p0, p1)]
            engs = (nc.sync, nc.scalar, nc.gpsimd)
            for i, (kind, arg) in enumerate(dma_ops):
                eng = engs[i % 3]
                if kind == "w":
                    p0, p1 = wchunks[arg]
                    w_c = wpool.tile(
                        [128, (p1 - p0) * WCOLS], f16, tag=f"w{arg}",
                        name=f"w_{arg}", bufs=1,
                    )
                    eng.dma_start(out=w_c[:, :], in_=wall[:, p0 * WCOLS : p1 * WCOLS])
                    for p in range(p0, p1):
                        wtiles[p] = (w_c, (p - p0) * WCOLS)
                elif kind == "bias":
                    bias_sb = wpool.tile(
                        [128, npair * 5], f32, name="bias_sb", tag="bias", bufs=1
                    )
                    eng.dma_start(out=bias_sb[:, :], in_=bias[:, :])
                elif kind == "x":
                    xt_p = xpool.tile([128, 2048], f16, tag="x", name=f"x_{arg}")
                    eng.dma_start(
                        out=xt_p[:, :], in_=xall[:, arg * 2048 : (arg + 1) * 2048]
                    )
                    xts[arg] = xt_p
                else:  # pair-0 x, split in two so L1(0) starts after 0.7us
                    half = 0 if kind == "x0a" else 1
                    xt_h = xpool.tile([128, 1024], f16, tag="x", name=kind)
                    eng.dma_start(
                        out=xt_h[:, :], in_=xall[:, half * 1024 : (half + 1) * 1024]
                    )
                    x0ab[half] = xt_h

            def xrhs(p, blk, c):
                if p == 0:
                    return x0ab[blk][:, c * BLK : (c + 1) * BLK]
                return xts[p][:, (2 * blk + c) * BLK : (2 * blk + c + 1) * BLK]

            # ---- diagonal duo/layer wavefront ---------------------------
            hcur = [None] * nduo

            def duo_relu(d, li, ph, dw):
                """bias+ReLU PSUM->SBUF for a whole duo. One op when the duo
                is single-expert and mid-pipeline; otherwise split into two
                ops across both engines (also shortens the fill/drain)."""
                h = hpool.tile([128, dw], f16, tag=f"h{li}", name=f"h{li}_{d}")
                pa, pb = 2 * d, min(2 * d + 1, npair - 1)
                edge = d == 0 or d == nduo - 1
                if dw == 1024 and duo_same[d] and not edge:
                    bap = bias_sb[:, 5 * pa + li : 5 * pa + li + 1]
                    if (d + li) % 2 == 0:
                        nc.vector.tensor_scalar(
                            h[:, :], ph[:, :], bap, 0.0, op0=add, op1=amax
                        )
                    else:
                        nc.scalar.activation(h[:, :], ph[:, :], Relu, bias=bap)
                else:
                    hh = dw // 2
                    ph_b = pb if dw == 1024 else pa
                    ba = bias_sb[:, 5 * pa + li : 5 * pa + li + 1]
                    bb = bias_sb[:, 5 * ph_b + li : 5 * ph_b + li + 1]
                    if (d + li) % 2 == 0:
                        nc.vector.tensor_scalar(
                            h[:, 0:hh], ph[:, 0:hh], ba, 0.0, op0=add, op1=amax
                        )
                        nc.scalar.activation(h[:, hh:dw], ph[:, hh:dw], Relu, bias=bb)
                    else:
                        nc.scalar.activation(h[:, 0:hh], ph[:, 0:hh], Relu, bias=ba)
                        nc.vector.tensor_scalar(
                            h[:, hh:dw], ph[:, hh:dw], bb, 0.0, op0=add, op1=amax
                        )
                return h

            for s in range(nduo + 5):
                for d in range(min(s, nduo - 1), max(0, s - 5) - 1, -1):
                    li = s - d
                    pairs = [p for p in (2 * d, 2 * d + 1) if p < npair]
                    dw = 512 * len(pairs)
                    if li == 0:
                        ph1 = ppool.tile([128, 1024], f32, tag="ph", name=f"ph1_{d}")
                        for pi, p in enumerate(pairs):
                            w, wo = wtiles[p]
                            for blk, colr in ((0, slice(0, 64)), (1, slice(64, 128))):
                                for c in range(2):
                                    nc.tensor.matmul(
                                        out=ph1[colr, pi * BLK : (pi + 1) * BLK],
                                        lhsT=w[:, wo + c * 64 : wo + (c + 1) * 64],
                                        rhs=xrhs(p, blk, c),
                                        start=(c == 0),
                                        stop=(c == 1),
                                    )
                        hcur[d] = duo_relu(d, 0, ph1, dw)
                    elif li <= 4:
                        ph = ppool.tile([128, 1024], f32, tag="ph", name=f"ph{li}_{d}")
                        for pi, p in enumerate(pairs):
                            w, wo = wtiles[p]
                            wc = wo + 128 * li
                            nc.tensor.matmul(
                                out=ph[:, pi * BLK : (pi + 1) * BLK],
                                lhsT=w[:, wc : wc + 128],
                                rhs=hcur[d][:, pi * BLK : (pi + 1) * BLK],
                                start=True,
                                stop=True,
                            )
                        hcur[d] = duo_relu(d, li, ph, dw)
                    else:
                        # L6 -> [36, dw] psum; copy halves on both engines
                        po = popool.tile([36, dw], f32, tag="po", name=f"po_{d}")
                        for pi, p in enumerate(pairs):
                            w, wo = wtiles[p]
                            nc.tensor.matmul(
                                out=po[:, pi * BLK : (pi + 1) * BLK],
                                lhsT=w[:, wo + 640 : wo + 676],
                                rhs=hcur[d][:, pi * BLK : (pi + 1) * BLK],
                                start=True,
                                stop=True,
                            )
                        o_d = opool.tile([36, dw], f16, tag="o", name=f"o_{d}")
                        hh = dw // 2
                        nc.vector.tensor_scalar(
                            o_d[:, 0:hh], po[:, 0:hh], 0.0, None, op0=add
                        )
                        nc.scalar.copy(o_d[:, hh:dw], po[:, hh:dw])
                        nc.sync.dma_start(
                            out=yt[:, 2 * d * BLK : 2 * d * BLK + dw], in_=o_d[:, :]
                        )

    nc.compile()
    return nc


def _get_program(npair: int, duo_same: tuple):
    key = (npair, duo_same)
    if key not in _PROGRAM_CACHE:
        _PROGRAM_CACHE[key] = _build_program(npair, duo_same)
    return _PROGRAM_CACHE[key]


def _prepare(state, rm_state, W1, b1, W2, b2, W3, b3, W4, b4, W5, b5, W6, b6):
    state = np.ascontiguousarray(np.asarray(state, dtype=np.float32))
    rm = np.asarray(rm_state).reshape(-1).astype(np.int64)
    Ws = [np.asarray(w, dtype=np.float32) for w in (W1, W2, W3, W4, W5, W6)]
    bs = [np.asarray(b, dtype=np.float32) for b in (b1, b2, b3, b4, b5, b6)]
    B = state.shape[0]
    X = state.reshape(B, D)

    # ---- host-side routing: stable sort rows by expert, pad groups so each
    # 1024-row pair is single-expert
    order = np.argsort(rm, kind="stable")
    counts = np.bincount(rm, minlength=E)
    caps = ((counts + PAIR - 1) // PAIR) * PAIR
    caps = np.maximum(caps, PAIR)  # empty groups still occupy one (zero) pair
    T0 = int(caps.sum())
    C = math.ceil(T0 / NCORES / PAIR) * PAIR
    T = NCORES * C
    caps[E - 1] += T - T0  # extend last group's padding to fill all cores
    base = np.zeros(E, dtype=np.int64)
    base[1:] = np.cumsum(caps)[:-1]
    csum = np.zeros(E, dtype=np.int64)
    csum[1:] = np.cumsum(counts)[:-1]
    sorted_expert = rm[order]
    pos_sorted = base[sorted_expert] + (np.arange(B) - csum[sorted_expert])

    Xp = np.zeros((T, D), np.float16)
    Xp[pos_sorted] = X[order].astype(np.float16)
    pair_expert = np.zeros(T // PAIR, np.int64)
    for e in range(E):
        pair_expert[base[e] // PAIR : (base[e] + caps[e]) // PAIR] = e

    W16 = [w.astype(np.float16) for w in Ws]
    npair = C // PAIR
    nduo = (npair + 1) // 2

    # duo d may use a single bias per ReLU only if single-expert on ALL cores
    pe_all = pair_expert.reshape(NCORES, npair)
    duo_same = []
    for dd in range(nduo):
        pa, pb = 2 * dd, min(2 * dd + 1, npair - 1)
        duo_same.append(bool((pe_all[:, pa] == pe_all[:, pb]).all()))
    duo_same = tuple(duo_same)

    # per-expert compact weight/bias panels, copied per pair below
    wex = np.zeros((E, 128, WCOLS), np.float16)
    bex = np.zeros((E, 128, 5), np.float32)
    for e in range(E):
        wex[e, :, 0:64] = W16[0][e, 0:128, :]
        wex[e, :, 64:128] = W16[0][e, 128:256, :]
        for li in range(4):
            wc = 128 + li * 128
            wex[e, 0:64, wc : wc + H] = W16[li + 1][e]
            wex[e, 64:128, wc + H : wc + 128] = W16[li + 1][e]
        wex[e, 0:64, 640 : 640 + A] = W16[5][e]
        wex[e, 64:128, 640 + A : 640 + 2 * A] = W16[5][e]
        for li in range(5):
            bex[e, 0:64, li] = bs[li][e]
            bex[e, 64:128, li] = bs[li][e]

    in_maps = []
    for core in range(NCORES):
        xt = Xp[core * C : (core + 1) * C].T  # [D, C] fp16 view
        pe = pair_expert[core * npair : (core + 1) * npair]

        xa = np.empty((128, npair * 2048), np.float16)
        for p in range(npair):
            for blk in range(2):
                src = xt[:, (2 * p + blk) * BLK : (2 * p + blk + 1) * BLK]
                dst = p * 2048 + blk * 1024
                xa[:, dst : dst + BLK] = src[0:128]
                xa[:, dst + BLK : dst + 2 * BLK] = src[128:256]

        wh = wex[pe].transpose(1, 0, 2).reshape(128, npair * WCOLS)
        bh = bex[pe].transpose(1, 0, 2).reshape(128, npair * 5)

        in_maps.append(
            {
                "xall": np.ascontiguousarray(xa),
                "wall": np.ascontiguousarray(wh),
                "bias": np.ascontiguousarray(bh),
            }
        )

    meta = dict(
        B=B, C=C, T=T, npair=npair, duo_same=duo_same, order=order,
        pos_sorted=pos_sorted, b6=bs[5], rm=rm,
    )
    return in_maps, meta


def _finalize(results, meta):
    """results: list (per core) of dicts with 'yt' [36, npair*BLK] arrays."""
    B, C, T, npair = (meta[k] for k in ("B", "C", "T", "npair"))
    Yp = np.zeros((T, A), np.float32)
    for core in range(NCORES):
        ytc = results[core]["yt"]
        for p in range(npair):
            cols = slice(p * BLK, (p + 1) * BLK)
            dst = core * C + 2 * p * BLK
            Yp[dst : dst + BLK] = ytc[0:A, cols].T
            Yp[dst + BLK : dst + 2 * BLK] = ytc[A : 2 * A, cols].T

    y = np.zeros((B, A), np.float32)
    y[meta["order"]] = Yp[meta["pos_sorted"]]
    y += meta["b6"][meta["rm"]]
    return y


def kernel(state, rm_state, W1, b1, W2, b2, W3, b3, W4, b4, W5, b5, W6, b6):
    global LAST_RESULTS
    from concourse.bass_utils import run_bass_kernel_spmd

    in_maps, meta = _prepare(
        state, rm_state, W1, b1, W2, b2, W3, b3, W4, b4, W5, b5, W6, b6
    )
    nc = _get_program(meta["npair"], meta["duo_same"])
    trace = bool(os.environ.get("KERNEL_TRACE"))
    res = run_bass_kernel_spmd(nc, in_maps, core_ids=list(range(NCORES)), trace=trace)
    LAST_RESULTS = res
    return _finalize(res.results, meta)
